# revision 13
# baseline (speedup 1.0000x reference)
"""GAT residual block (nn_GATResBlock) on 8 Trainium2 NeuronCores.

Strategy
--------
- Shard destination nodes (and their incoming edges) across the 8 cores;
  each core owns a contiguous range of 6250 dst nodes.
- Host-side graph preprocessing (sanctioned by the sharding hint): sort each
  core's edges by dst block (128 dsts per block), build padded per-block edge
  lists and int16 gather-index arrays.
- Algebraic folds: a_src = x @ (W.T @ att_src-expanded) so the attention
  logits come out of the same projection matmul; segment-softmax max-trick is
  dropped (logits are bounded, softmax is shift invariant) and the softmax is
  normalized at the *node* level: agg = (sum ex*xp[src]) / (sum ex), so no
  per-edge alpha is ever materialized.
- Device per core: one replicated projection pass builds a DRAM node table
  T1[row] = [xp | a_src]; per dst-block, dma_gather fetches the rows of the
  block's source nodes, a second small gather broadcasts a_dst from a local
  table, a one-hot (edge,dst) selection matrix is built with iota/is_equal and
  a PSUM-accumulated matmul reduces weighted messages + softmax denominators
  in one pass. Epilogue divides, adds the skip projection and applies ELU.
- int16 gather indices only span 32768 rows, so the node table is gathered by
  two calls: rows [0, 32768) ("A") and [32768, ...) ("B"); the host splits
  each block's edge list accordingly.
"""

import sys
import types

sys.path.insert(0, "/opt/trn_rl_repo")

import numpy as np


# ---------------------------------------------------------------------------
# NTFF profile hook (missing antenv.axon_hooks in this image). Needed only
# when tracing; harmless otherwise.
def _install_ntff_hook():
    if "antenv.axon_hooks" in sys.modules:
        return
    try:
        hooks = types.ModuleType("antenv.axon_hooks")
        _h = [None]
        hooks.set_axon_ntff_profile_hook = lambda h: _h.__setitem__(0, h)
        hooks.get_axon_ntff_profile_hook = lambda: _h[0]
        sys.modules["antenv.axon_hooks"] = hooks
        import antenv

        antenv.axon_hooks = hooks
        from trn_agent_boot.trn_boot import _ntff_profile_via_ctypes

        hooks.set_axon_ntff_profile_hook(
            _ntff_profile_via_ctypes("/opt/axon/libaxon_pjrt.so")
        )
    except Exception:
        pass


_install_ntff_hook()

from concourse import bacc, bass, mybir, tile  # noqa: E402
from concourse.bass_utils import run_bass_kernel_spmd  # noqa: E402

F32 = mybir.dt.float32
I16 = mybir.dt.int16
ALU = mybir.AluOpType
ACTF = mybir.ActivationFunctionType

P = 128
NEG_SLOPE = 0.2
NEG_BIG = -1.0e30


class Cfg:
    def __init__(self, N=50000, IN=128, H=4, C=32, E=800000, NC=8, SPLIT=32768,
                 TA=None, TB=None):
        self.N, self.IN, self.H, self.C, self.E, self.NC = N, IN, H, C, E, NC
        self.HC = H * C
        assert self.HC == 128 and IN == 128
        assert N % NC == 0
        self.NLOC = N // NC                      # owned dst nodes per core
        self.NBLK = (self.NLOC + P - 1) // P     # dst blocks per core
        self.NLOCP = self.NBLK * P               # padded local nodes
        self.SPLIT = SPLIT                       # int16 A/B table split
        nrows = 1 + N + 1                        # PAD_A + nodes + PAD_B
        self.NR = ((nrows + P - 1) // P) * P     # node-table rows (padded)
        assert self.NR - SPLIT <= 32768
        self.PAD_B = N + 1                       # table row of the B pad
        self.T2ROWS = ((self.NLOCP + 1 + P - 1) // P) * P
        self.ROWW = 192                          # T1 row: xp(128)+a_src(4)+pad
        self.TA, self.TB = TA, TB                # edge tiles per block (A/B)

    @property
    def T(self):
        return self.TA + self.TB


# ---------------------------------------------------------------------------
# Host-side preprocessing: edge partitioning + gather index construction.


def _wrap_idx(arr):
    """[K*128] edge-slot array -> [128, K*8] int16 'wrapped' index layout
    (index i lives at [i % 16, i // 16], replicated across the 8 groups)."""
    k16 = arr.reshape(-1, 16).T.astype(np.int16)  # [16, K*8]
    return np.tile(k16, (8, 1))                   # [128, K*8]


def preprocess(cfg, edge_index):
    """Build per-core gather index arrays from the (2, E) edge list."""
    src = np.asarray(edge_index[0], dtype=np.int64)
    dst = np.asarray(edge_index[1], dtype=np.int64)
    core = dst // cfg.NLOC
    dstl = dst - core * cfg.NLOC
    blk = dstl // P
    srow = src + 1                                # +1: table row 0 is PAD_A
    isB = (srow >= cfg.SPLIT).astype(np.int64)

    order = np.lexsort((srow, isB, blk, core))
    core_s, blk_s, isB_s = core[order], blk[order], isB[order]
    srow_s, dstl_s = srow[order], dstl[order]

    gid = ((core_s * cfg.NBLK) + blk_s) * 2 + isB_s
    ngroups = cfg.NC * cfg.NBLK * 2
    counts = np.bincount(gid, minlength=ngroups)
    starts = np.concatenate(([0], np.cumsum(counts)[:-1]))
    rank = np.arange(len(gid)) - starts[gid]

    cA = counts.reshape(cfg.NC, cfg.NBLK, 2)[:, :, 0]
    cB = counts.reshape(cfg.NC, cfg.NBLK, 2)[:, :, 1]
    if cfg.TA is None:
        cfg.TA = max(1, int(-(-cA.max() // P)))
        cfg.TB = max(1, int(-(-cB.max() // P)))
    TA, TB, T = cfg.TA, cfg.TB, cfg.T
    assert cA.max() <= TA * P and cB.max() <= TB * P

    idxA = np.zeros((cfg.NC, cfg.NBLK, TA * P), dtype=np.int64)      # pad: row 0
    idxB = np.full((cfg.NC, cfg.NBLK, TB * P), cfg.PAD_B - cfg.SPLIT,
                   dtype=np.int64)
    idxD = np.full((cfg.NC, cfg.NBLK, T * P), cfg.NLOCP, dtype=np.int64)
    dloc = np.zeros((cfg.NC, cfg.NBLK, T * P), dtype=np.float32)

    a = isB_s == 0
    idxA[core_s[a], blk_s[a], rank[a]] = srow_s[a]
    idxD[core_s[a], blk_s[a], rank[a]] = dstl_s[a]
    dloc[core_s[a], blk_s[a], rank[a]] = (dstl_s[a] - blk_s[a] * P)
    b = ~a
    idxB[core_s[b], blk_s[b], rank[b]] = srow_s[b] - cfg.SPLIT
    idxD[core_s[b], blk_s[b], TA * P + rank[b]] = dstl_s[b]
    dloc[core_s[b], blk_s[b], TA * P + rank[b]] = (dstl_s[b] - blk_s[b] * P)

    per_core = []
    for c in range(cfg.NC):
        wA = np.concatenate([_wrap_idx(idxA[c, b2]) for b2 in range(cfg.NBLK)],
                            axis=1)
        wB = np.concatenate([_wrap_idx(idxB[c, b2]) for b2 in range(cfg.NBLK)],
                            axis=1)
        wD = np.concatenate([_wrap_idx(idxD[c, b2]) for b2 in range(cfg.NBLK)],
                            axis=1)
        # dloc DRAM layout [128, NBLK*T]: [p, b*T + t] = slot (b, t, p)
        dl = dloc[c].reshape(cfg.NBLK, T, P).transpose(2, 0, 1).reshape(P, -1)
        per_core.append(dict(idxA=np.ascontiguousarray(wA),
                             idxB=np.ascontiguousarray(wB),
                             idxD=np.ascontiguousarray(wD),
                             dloc=np.ascontiguousarray(dl)))
    return per_core


def make_weights(cfg, W, att_src, att_dst, bias, skip_W, skip_b):
    """Fold attention vectors into the projection weights."""
    H, C, IN = cfg.H, cfg.C, cfg.IN
    A_s = np.zeros((IN, H), dtype=np.float32)
    A_d = np.zeros((IN, H), dtype=np.float32)
    for h in range(H):
        # a_src[n,h] = sum_c xp[n,h*C+c]*att_src[h,c] = x @ (W[h*C:+C].T @ att)
        A_s[:, h] = W[h * C:(h + 1) * C, :].T @ att_src[0, h]
        A_d[:, h] = W[h * C:(h + 1) * C, :].T @ att_dst[0, h]
    Wcat = np.concatenate([W.T, A_s, A_d], axis=1).astype(np.float32)  # [IN,136]
    Wsk = np.concatenate([skip_W.T, A_d], axis=1).astype(np.float32)   # [IN,132]
    bias2 = np.tile((bias + skip_b).astype(np.float32)[None, :], (P, 1))
    return Wcat, Wsk, bias2


def make_inputs(cfg, x, edge_index, W, att_src, att_dst, bias, skip_W, skip_b):
    per_core_idx = preprocess(cfg, edge_index)
    Wcat, Wsk, bias2 = make_weights(cfg, W, att_src, att_dst, bias, skip_W,
                                    skip_b)
    xT = np.zeros((cfg.IN, cfg.NR), dtype=np.float32)
    xT[:, 1:1 + cfg.N] = np.asarray(x, dtype=np.float32).T
    iota = np.tile(np.arange(P, dtype=np.float32)[None, :], (P, 1))
    negr = np.full((1, 4), NEG_BIG, dtype=np.float32)

    in_maps = []
    for c in range(cfg.NC):
        xTl = np.zeros((cfg.IN, cfg.NLOCP), dtype=np.float32)
        xTl[:, :cfg.NLOC] = np.asarray(
            x[c * cfg.NLOC:(c + 1) * cfg.NLOC], dtype=np.float32).T
        m = dict(xT=xT, xTl=np.ascontiguousarray(xTl), Wcat=Wcat, Wsk=Wsk,
                 bias2=bias2, iota=iota, negr=negr, **per_core_idx[c])
        in_maps.append(m)
    return in_maps


# ---------------------------------------------------------------------------
# Device program.


def build_program(cfg, debug_level=99):
    """debug_level: 1=prologue only, 2=+gathers, 3=+edge math, 4=+matmul,
    99=full."""
    nc = bacc.Bacc(None)
    TA, TB, T = cfg.TA, cfg.TB, cfg.T
    NBLK, NR, ROWW = cfg.NBLK, cfg.NR, cfg.ROWW

    xT = nc.declare_dram_parameter("xT", [cfg.IN, NR], F32, isOutput=False)
    xTl = nc.declare_dram_parameter("xTl", [cfg.IN, cfg.NLOCP], F32,
                                    isOutput=False)
    Wcat = nc.declare_dram_parameter("Wcat", [cfg.IN, 136], F32, isOutput=False)
    Wsk = nc.declare_dram_parameter("Wsk", [cfg.IN, 132], F32, isOutput=False)
    bias2 = nc.declare_dram_parameter("bias2", [P, 128], F32, isOutput=False)
    iota = nc.declare_dram_parameter("iota", [P, P], F32, isOutput=False)
    negr = nc.declare_dram_parameter("negr", [1, 4], F32, isOutput=False)
    idxA = nc.declare_dram_parameter("idxA", [P, NBLK * TA * 8], I16,
                                     isOutput=False)
    idxB = nc.declare_dram_parameter("idxB", [P, NBLK * TB * 8], I16,
                                     isOutput=False)
    idxD = nc.declare_dram_parameter("idxD", [P, NBLK * T * 8], I16,
                                     isOutput=False)
    dloc = nc.declare_dram_parameter("dloc", [P, NBLK * T], F32,
                                     isOutput=False)
    out = nc.declare_dram_parameter("out", [cfg.NLOCP, 128], F32,
                                    isOutput=True)

    T1 = nc.dram_tensor("T1", [NR, ROWW], F32)
    T2L = nc.dram_tensor("T2L", [cfg.T2ROWS, 64], F32)

    with tile.TileContext(nc) as tc:
        with (
            tc.tile_pool(name="const", bufs=1) as cpool,
            tc.tile_pool(name="prol", bufs=4) as prol,
            tc.tile_pool(name="pp", bufs=2, space="PSUM") as pp,
            tc.tile_pool(name="main", bufs=2) as mp,
            tc.tile_pool(name="acc", bufs=2, space="PSUM") as ap,
            tc.tile_pool(name="epi", bufs=2) as ep,
        ):
            # ---- constants ----
            iota_sb = cpool.tile([P, P], F32)
            nc.sync.dma_start(out=iota_sb[:], in_=iota[:])
            wcat_sb = cpool.tile([P, 136], F32)
            nc.sync.dma_start(out=wcat_sb[:], in_=Wcat[:])
            wsk_sb = cpool.tile([P, 132], F32)
            nc.sync.dma_start(out=wsk_sb[:], in_=Wsk[:])
            bias_sb = cpool.tile([P, 128], F32)
            nc.sync.dma_start(out=bias_sb[:], in_=bias2[:])
            negr_sb = cpool.tile([1, 4], F32)
            nc.sync.dma_start(out=negr_sb[:], in_=negr[:])
            idxA_sb = cpool.tile([P, NBLK * TA * 8], I16)
            nc.sync.dma_start(out=idxA_sb[:], in_=idxA[:])
            idxB_sb = cpool.tile([P, NBLK * TB * 8], I16)
            nc.sync.dma_start(out=idxB_sb[:], in_=idxB[:])
            idxD_sb = cpool.tile([P, NBLK * T * 8], I16)
            nc.sync.dma_start(out=idxD_sb[:], in_=idxD[:])
            dloc_sb = cpool.tile([P, NBLK * T], F32)
            nc.sync.dma_start(out=dloc_sb[:], in_=dloc[:])
            skip_sb = cpool.tile([P, NBLK * 128], F32)

            # ---- phase 1: build global node table T1 = [xp | a_src] ----
            for i in range(NR // P):
                xt = prol.tile([P, P], F32)
                nc.sync.dma_start(out=xt[:], in_=xT[:, i * P:(i + 1) * P])
                ps = pp.tile([P, 136], F32)
                nc.tensor.matmul(out=ps[:], lhsT=xt[:], rhs=wcat_sb[:],
                                 start=True, stop=True)
                st = prol.tile([P, 136], F32)
                nc.vector.tensor_copy(out=st[:], in_=ps[:])
                nc.sync.dma_start(out=T1[i * P:(i + 1) * P, 0:136], in_=st[:])
            # pad rows: a_src = NEG_BIG so padded edges contribute ex = 0
            nc.sync.dma_start(out=T1[0:1, 128:132], in_=negr_sb[:])
            nc.sync.dma_start(out=T1[cfg.PAD_B:cfg.PAD_B + 1, 128:132],
                              in_=negr_sb[:])

            # ---- phase 2: local skip projection + a_dst table ----
            for j in range(NBLK):
                xl = prol.tile([P, P], F32)
                nc.sync.dma_start(out=xl[:], in_=xTl[:, j * P:(j + 1) * P])
                ps2 = pp.tile([P, 132], F32)
                nc.tensor.matmul(out=ps2[:], lhsT=xl[:], rhs=wsk_sb[:],
                                 start=True, stop=True)
                nc.vector.tensor_tensor(out=skip_sb[:, j * P:(j + 1) * P],
                                        in0=ps2[:, 0:128], in1=bias_sb[:],
                                        op=ALU.add)
                ad = prol.tile([P, 4], F32)
                nc.vector.tensor_copy(out=ad[:], in_=ps2[:, 128:132])
                nc.sync.dma_start(out=T2L[j * P:(j + 1) * P, 0:4], in_=ad[:])
            nc.sync.dma_start(out=T2L[cfg.NLOCP:cfg.NLOCP + 1, 0:4],
                              in_=negr_sb[:])

            # ---- phase 3: per-dst-block edge processing ----
            for b in range(NBLK):
                if debug_level < 2:
                    nc.sync.dma_start(out=out[b * P:(b + 1) * P, :],
                                      in_=bias_sb[:])
                    continue
                G1 = mp.tile([P, T, ROWW], F32)
                nc.gpsimd.dma_gather(
                    out_ap=G1[:, 0:TA, :],
                    in_ap=T1[:],
                    idxs_ap=idxA_sb[:, b * TA * 8:(b + 1) * TA * 8],
                    num_idxs=TA * P,
                    num_idxs_reg=TA * P,
                    elem_size=ROWW,
                    single_packet=False,
                )
                nc.gpsimd.dma_gather(
                    out_ap=G1[:, TA:T, :],
                    in_ap=T1[cfg.SPLIT:, :],
                    idxs_ap=idxB_sb[:, b * TB * 8:(b + 1) * TB * 8],
                    num_idxs=TB * P,
                    num_idxs_reg=TB * P,
                    elem_size=ROWW,
                    single_packet=False,
                )
                G2 = mp.tile([P, T, 64], F32)
                nc.gpsimd.dma_gather(
                    out_ap=G2[:],
                    in_ap=T2L[:],
                    idxs_ap=idxD_sb[:, b * T * 8:(b + 1) * T * 8],
                    num_idxs=T * P,
                    num_idxs_reg=T * P,
                    elem_size=64,
                    single_packet=False,
                )
                if debug_level < 3:
                    st3 = mp.tile([P, 128], F32)
                    nc.vector.tensor_copy(out=st3[:], in_=G1[:, 0, 0:128])
                    nc.vector.tensor_copy(out=st3[:, 0:64], in_=G2[:, 0, :])
                    nc.sync.dma_start(out=out[b * P:(b + 1) * P, :],
                                      in_=st3[:])
                    continue
                # edge logits -> ex = exp(leaky_relu(a_src + a_dst))
                elog = mp.tile([P, T, 4], F32)
                nc.vector.tensor_tensor(out=elog[:], in0=G1[:, :, 128:132],
                                        in1=G2[:, :, 0:4], op=ALU.add)
                el2 = mp.tile([P, T, 4], F32)
                nc.vector.scalar_tensor_tensor(out=el2[:], in0=elog[:],
                                               scalar=NEG_SLOPE, in1=elog[:],
                                               op0=ALU.mult, op1=ALU.max)
                el3 = mp.tile([P, T, 4], F32)
                nc.vector.tensor_scalar_max(out=el3[:], in0=el2[:],
                                            scalar1=-87.0)
                ex = mp.tile([P, T, 4], F32)
                nc.scalar.activation(out=ex[:], in_=el3[:], func=ACTF.Exp)
                # V = [ex * xp | ex]
                V = mp.tile([P, T, 132], F32)
                nc.vector.tensor_tensor(
                    out=V[:, :, 0:128].rearrange("p t (h c) -> p t h c", c=32),
                    in0=G1[:, :, 0:128].rearrange("p t (h c) -> p t h c", c=32),
                    in1=ex[:, :, :, None].to_broadcast([P, T, 4, 32]),
                    op=ALU.mult,
                )
                nc.vector.tensor_copy(out=V[:, :, 128:132], in_=ex[:])
                # one-hot selection matrix S[e, d] = (dloc[e] == d)
                S = mp.tile([P, T, P], F32)
                nc.vector.tensor_tensor(
                    out=S[:],
                    in0=dloc_sb[:, b * T:(b + 1) * T, None].to_broadcast(
                        [P, T, P]),
                    in1=iota_sb[:, None, :].to_broadcast([P, T, P]),
                    op=ALU.is_equal,
                )
                if debug_level < 4:
                    st4 = mp.tile([P, 128], F32)
                    nc.vector.tensor_copy(out=st4[:], in_=S[:, 0, :])
                    nc.vector.tensor_copy(out=st4[:], in_=V[:, 0, 0:128])
                    nc.sync.dma_start(out=out[b * P:(b + 1) * P, :],
                                      in_=st4[:])
                    continue
                acc = ap.tile([P, 132], F32)
                for t in range(T):
                    nc.tensor.matmul(out=acc[:], lhsT=S[:, t, :],
                                     rhs=V[:, t, :], start=(t == 0),
                                     stop=(t == T - 1))
                if debug_level < 99:
                    st5 = mp.tile([P, 128], F32)
                    nc.vector.tensor_copy(out=st5[:], in_=acc[:, 0:128])
                    nc.sync.dma_start(out=out[b * P:(b + 1) * P, :],
                                      in_=st5[:])
                    continue
                # epilogue: divide, + skip, ELU
                dn = ep.tile([P, 4], F32)
                # 1e-6 (not the reference's 1e-16) keeps the reciprocal input
                # inside the scalar-engine valid range; relative effect on
                # real denominators is < 1e-5.
                nc.vector.tensor_scalar_add(out=dn[:], in0=acc[:, 128:132],
                                            scalar1=1e-6)
                rcp = ep.tile([P, 4], F32)
                nc.vector.reciprocal(out=rcp[:], in_=dn[:])
                y = ep.tile([P, 128], F32)
                nc.vector.tensor_tensor(
                    out=y[:].rearrange("p (h c) -> p h c", c=32),
                    in0=acc[:, 0:128].rearrange("p (h c) -> p h c", c=32),
                    in1=rcp[:, :, None].to_broadcast([P, 4, 32]),
                    op=ALU.mult,
                )
                y2 = ep.tile([P, 128], F32)
                nc.vector.tensor_tensor(out=y2[:], in0=y[:],
                                        in1=skip_sb[:, b * P:(b + 1) * P],
                                        op=ALU.add)
                # elu(v) = max(v,0) + exp(min(v,0)) - 1
                mn = ep.tile([P, 128], F32)
                nc.vector.tensor_scalar_min(out=mn[:], in0=y2[:], scalar1=0.0)
                e1 = ep.tile([P, 128], F32)
                nc.scalar.activation(out=e1[:], in_=mn[:], func=ACTF.Exp)
                mx = ep.tile([P, 128], F32)
                nc.vector.tensor_scalar_max(out=mx[:], in0=y2[:], scalar1=0.0)
                yo = ep.tile([P, 128], F32)
                nc.vector.scalar_tensor_tensor(out=yo[:], in0=mx[:],
                                               scalar=-1.0, in1=e1[:],
                                               op0=ALU.add, op1=ALU.add)
                nc.sync.dma_start(out=out[b * P:(b + 1) * P, :], in_=yo[:])

    nc.compile()
    return nc


# ---------------------------------------------------------------------------
# Public entry point.

_CACHE = {}


def _get_program(cfg):
    key = (cfg.N, cfg.E, cfg.NC, cfg.TA, cfg.TB)
    if key not in _CACHE:
        _CACHE[key] = build_program(cfg)
    return _CACHE[key]


def run_full(inputs, trace=False, **spmd_kwargs):
    cfg = Cfg()
    in_maps = make_inputs(cfg, **{k: np.asarray(v) for k, v in inputs.items()})
    nc = _get_program(cfg)
    res = run_bass_kernel_spmd(nc, in_maps, list(range(cfg.NC)), trace=trace,
                               **spmd_kwargs)
    outs = [res.results[c]["out"][:cfg.NLOC] for c in range(cfg.NC)]
    return np.concatenate(outs, axis=0).astype(np.float32), res


def kernel(x, edge_index, W, att_src, att_dst, bias, skip_W, skip_b):
    out, _ = run_full(dict(x=x, edge_index=edge_index, W=W, att_src=att_src,
                           att_dst=att_dst, bias=bias, skip_W=skip_W,
                           skip_b=skip_b))
    return out
